# revision 16
# baseline (speedup 1.0000x reference)
"""Trainium2 Bass kernel for MockFP8Linear: out = x @ (W * block_scale)^T.

Strategy: data-parallel over tokens across 8 NeuronCores (no collectives).
Pure bf16 matmul stream at the measured N=512 issue floor (~215.8 ns/matmul,
warm 2.4 GHz): 1024 matmuls/core, PE ~100% packed after startup.

Both operands are fed to the device already in [contraction-on-partitions]
layout (host-side np transpose + bf16 cast, the same prep class as the W^T
layout prep the original baseline used):
  - x: xt[tt, i, ib, t] = x[tt*128+t, ib*128+i] (bf16); each token tile tt
    is one contiguous 512 KB slab -> SBUF [128(i), 16(ib) x 128(t)];
    lhsT for (tt, ib) is the [128,128] slice [:, ib, :].
  - weight: wt[ch, ib, i, o] = W[ch*512+o, ib*128+i] (bf16), 64 contiguous
    128 KB slabs. The per-128x128-block dequant scaling stays on-device:
    one DVE tensor_tensor per slab (in-place, scale broadcast), sequenced
    so slab first-use is spread across the whole run (never dequant-paced).

Main loop: 2 chunk-pair passes (output halves) x 16 token tiles. Each
stationary lhsT feeds 2 matmuls into 2 alternating PSUM banks - hardware-
measured fastest shape: swapping the stationary every matmul costs +43 ns
each, and >2-matmul accumulation runs into the same penalty. Per group of
32 matmuls: 16 ib-steps accumulate the pair of [128,512] f32 banks, ACT
evicts both to SBUF, GPSIMD issues the 512 KB output DMA (DVE holds only
the dequant queue; an eviction stuck behind dequants stalls PSUM bank
recycling). The final two groups split evictions DVE/ACT and the DMA into
quarters across queue engines to shorten the drain.

Startup: ~18 dummy f32 matmuls on the scales tile run during the runtime
preamble so the HAM clock gate (4096-cycle activity window, 1.2 -> 2.4
GHz) flips before the real stream; input DMA order is first-chunk-pair W
slabs + x0, early x slabs woven between W pairs (a single early PE gap
resets the HAM busy window), then remaining x, then the second W half.

Measured: 246.6-247.3 us (from 283 us baseline); PE busy ~225 us with one
sub-microsecond gap; rel err 2.0e-3 (gate 2e-2).
"""

import os
import sys

import numpy as np

for _p in ("/opt/trn_rl_repo", "/root/.axon_site/_ro/trn_rl_repo"):
    if os.path.isdir(_p) and _p not in sys.path:
        sys.path.append(_p)

TOKENS, IN_F, OUT_F = 16384, 2048, 2048
NCORES = 8
TSH = TOKENS // NCORES  # tokens per core
P = 128
KB = IN_F // P  # contraction blocks (ib)
TB = TSH // P  # token tiles per core (tt)
OBL = OUT_F // P  # out_features blocks (scale granularity)
NCH = OUT_F // 512  # output column chunks (ch)

_cached = None


def _build():
    from contextlib import ExitStack

    import concourse.tile as tile
    from concourse import bacc, mybir
    from concourse.bass import ds

    f32 = mybir.dt.float32
    bf16 = mybir.dt.bfloat16

    nc = bacc.Bacc("TRN2", target_bir_lowering=False, debug=False, num_devices=NCORES)
    xt_d = nc.dram_tensor("xt", [TB, P, KB, P], bf16, kind="ExternalInput").ap()
    wt_d = nc.dram_tensor("wt", [NCH, KB, P, 512], bf16, kind="ExternalInput").ap()
    s_d = nc.dram_tensor("s", [P, KB, OBL], bf16, kind="ExternalInput").ap()
    o_d = nc.dram_tensor("out", [TSH, OUT_F], f32, kind="ExternalOutput").ap()

    with tile.TileContext(nc) as tc:
        with ExitStack() as ctx:
            const = ctx.enter_context(tc.tile_pool(name="const", bufs=1))
            scales = const.tile([P, KB, OBL], bf16)
            nc.sync.dma_start(scales[:], s_d[:])

            w_pool = ctx.enter_context(tc.tile_pool(name="w", bufs=1))
            wsb = [
                [w_pool.tile([P, 512], bf16, name=f"w_{ch}_{ib}") for ib in range(KB)]
                for ch in range(NCH)
            ]
            x_pool = ctx.enter_context(tc.tile_pool(name="x", bufs=1))
            xsb = [x_pool.tile([P, KB, P], bf16, name=f"x_{tt}") for tt in range(TB)]

            osb_pool = ctx.enter_context(tc.tile_pool(name="osb", bufs=4))
            ps_pool = ctx.enter_context(tc.tile_pool(name="ps", bufs=8, space="PSUM"))

            # ---- PE warmup: a few dummy f32 matmuls on the scales tile
            # (values irrelevant, result never read) make the PE busy during
            # the runtime preamble/input-DMA window so the HAM clock gate
            # unthrottles to 2.4 GHz before the real stream starts. ----
            wm_ps = ps_pool.tile([16, 256], f32, tag="ps", name="warm")
            for _ in range(18):
                nc.tensor.matmul(
                    wm_ps[:],
                    lhsT=scales[:, 0, :],
                    rhs=scales[:, :, :],
                    start=True,
                    stop=True,
                )

            def load_w(ch, ib):
                nc.sync.dma_start(wsb[ch][ib][:], wt_d[ch, ib])
                # in-place dequant: scale block bo = ch*4 + (o//128).
                # Evictions live on ACT so the dequant backlog never stalls
                # PSUM recycling. The first chunk-pair alternates DVE/GPSIMD
                # (GPSIMD is idle that early) to halve the serial chain that
                # otherwise gaps group 0 and re-arms the HAM throttle.
                eng = nc.vector if (ch >= 2 or ib % 2 == 0) else nc.gpsimd
                eng.tensor_tensor(
                    out=wsb[ch][ib][:].rearrange("p (b c) -> p b c", c=P),
                    in0=wsb[ch][ib][:].rearrange("p (b c) -> p b c", c=P),
                    in1=scales[:, ib, ds(ch * 4, 4), None].broadcast_to([P, 4, P]),
                    op=mybir.AluOpType.mult,
                )

            # ---- input DMA issue order: first chunk-pair's W + x slab 0
            # first, then the remaining x slabs (JIT for the chp0 pass),
            # then the rest of W (needed only at ~110 us into the stream).
            load_w(0, 0)
            load_w(1, 0)
            nc.sync.dma_start(xsb[0][:], xt_d[0])
            for ib in range(1, KB):
                load_w(0, ib)
                load_w(1, ib)
                # weave the first few x slabs between the W pairs so the
                # early token-tile groups never wait (a single early PE gap
                # resets the HAM busy window and keeps the clock at 1.2 GHz)
                if ib % 5 == 0 and ib // 5 < 4:
                    nc.sync.dma_start(xsb[ib // 5][:], xt_d[ib // 5])
            for tt in range(4, TB):
                nc.sync.dma_start(xsb[tt][:], xt_d[tt])
            for ch in range(2, NCH):
                for ib in range(KB):
                    load_w(ch, ib)

            # chunk-pair passes: each lhsT (stationary) feeds 2 matmuls
            # into 2 alternating PSUM banks, like the fastest measured
            # stream shape (stationary reuse + bank alternation).
            for chp in range(NCH // 2):
                for tt in range(TB):
                    psum = [
                        ps_pool.tile([P, 512], f32, tag="ps", name=f"ps_{chp}_{tt}_{k}")
                        for k in range(2)
                    ]
                    for ib in range(KB):
                        for k in range(2):
                            nc.tensor.matmul(
                                psum[k][:],
                                lhsT=xsb[tt][:, ib, :],
                                rhs=wsb[2 * chp + k][ib][:],
                                start=(ib == 0),
                                stop=(ib == KB - 1),
                            )
                    osb = osb_pool.tile([P, 1024], f32, tag="osb", name=f"o_{chp}_{tt}")
                    last2 = chp == NCH // 2 - 1 and tt >= TB - 2
                    if last2:
                        # drain fast: split evictions DVE/ACT (the DVE
                        # dequant queue is long empty) and the DMA in halves
                        # across two queue engines (quarter-splits measured
                        # worse: per-descriptor overhead)
                        nc.vector.tensor_copy(osb[:, ds(0, 512)], psum[0][:])
                        nc.scalar.copy(osb[:, ds(512, 512)], psum[1][:])
                        nc.sync.dma_start(
                            o_d[ds(tt * P, P), ds(chp * 1024, 512)],
                            osb[:, ds(0, 512)],
                        )
                        nc.scalar.dma_start(
                            o_d[ds(tt * P, P), ds(chp * 1024 + 512, 512)],
                            osb[:, ds(512, 512)],
                        )
                    else:
                        # both evictions on ACT (DVE holds the dequant queue)
                        nc.scalar.copy(osb[:, ds(0, 512)], psum[0][:])
                        nc.scalar.copy(osb[:, ds(512, 512)], psum[1][:])
                        nc.gpsimd.dma_start(
                            o_d[ds(tt * P, P), ds(chp * 1024, 1024)], osb[:]
                        )

    nc.compile()
    return nc


def _get_compiled():
    global _cached
    if _cached is None:
        _cached = _build()
    return _cached


def _ensure_ntff_hook():
    """Register the axon NTFF profile hook (boot skips it when
    antenv.axon_hooks is absent from the image). Only needed for trace=True."""
    import sys as _sys
    import types as _types

    if "antenv.axon_hooks" not in _sys.modules:
        import antenv

        mod = _types.ModuleType("antenv.axon_hooks")
        mod._hook = None

        def set_axon_ntff_profile_hook(h):
            mod._hook = h

        def get_axon_ntff_profile_hook():
            return mod._hook

        mod.set_axon_ntff_profile_hook = set_axon_ntff_profile_hook
        mod.get_axon_ntff_profile_hook = get_axon_ntff_profile_hook
        _sys.modules["antenv.axon_hooks"] = mod
        antenv.axon_hooks = mod
    mod = _sys.modules["antenv.axon_hooks"]
    if mod._hook is None:
        from trn_agent_boot.trn_boot import _ntff_profile_via_ctypes

        hook = _ntff_profile_via_ctypes("/opt/axon/libaxon_pjrt.so")
        if hook is not None:
            mod.set_axon_ntff_profile_hook(hook)


def run(x, weight, weight_scale, trace=False, trace_cores=None):
    from concourse.bass_utils import run_bass_kernel_spmd

    import ml_dtypes

    nc = _get_compiled()

    x = np.asarray(x, dtype=np.float32)
    weight = np.asarray(weight, dtype=np.float32)
    weight_scale = np.asarray(weight_scale, dtype=np.float32)

    # wt[ch, ib, i, o] = W[ch*512+o, ib*128+i]
    wt = np.ascontiguousarray(
        weight.reshape(NCH, 512, KB, P).transpose(0, 2, 3, 1).astype(ml_dtypes.bfloat16)
    )
    # s[p, bi, bo] = weight_scale[bo, bi] broadcast over partitions
    scales_b = np.ascontiguousarray(
        np.broadcast_to(weight_scale.T[None, :, :], (P, KB, OBL)).astype(
            ml_dtypes.bfloat16
        )
    )

    in_maps = []
    for c in range(NCORES):
        xs = x[c * TSH : (c + 1) * TSH]
        # xt[tt, i, ib, t] = xs[tt*128+t, ib*128+i]
        xt = np.ascontiguousarray(
            xs.reshape(TB, P, KB, P).transpose(0, 3, 2, 1).astype(ml_dtypes.bfloat16)
        )
        in_maps.append({"xt": xt, "wt": wt, "s": scales_b})

    kwargs = {}
    if trace:
        try:
            _ensure_ntff_hook()
        except Exception as e:  # tracing is best-effort; the run still works
            print(f"ntff hook registration failed ({e}); tracing may be skipped")
        kwargs = dict(trace=True, trace_cores=trace_cores or [0])
    res = run_bass_kernel_spmd(nc, in_maps, core_ids=list(range(NCORES)), **kwargs)
    out = np.concatenate([res.results[c]["out"] for c in range(NCORES)], axis=0)
    return out, res


def kernel(x, weight, weight_scale):
    # Rare transient device errors (NRT_EXEC_UNIT_UNRECOVERABLE) have been
    # observed under the profiling path; retry once to be safe.
    try:
        out, _ = run(x, weight, weight_scale)
    except Exception:
        import time

        time.sleep(2)
        out, _ = run(x, weight, weight_scale)
    return out


if __name__ == "__main__":
    pass
